# revision 3
# baseline (speedup 1.0000x reference)
"""ChannelAtten (XCA-style channel attention) on 8 TRN2 NeuronCores — v4.

Sharding: (batch, H-half) -> 8 shards. Per core, bf16 pipeline:
- qkv 1x1 conv on PE (bf16), ACT evacuates PSUM->SBUF bf16 with bias.
- q/k channels permuted host-side into per-head [q_h|k_h] interleave so the
  gram matmul computes qq/qk/kq/kk per head in one 96x96 block (sumsq = diag).
- depthwise 3x3 split across engines: t0 on DVE (ts products + tt adds, 4x/2x
  bf16 modes), t1 kh=1 on PE + kh0/kh2 products on ACT+DVE, t2 on Pool (stt
  chain), t3/t4 (v) fully on PE with fp32 PSUM accumulation.
- v stays in SBUF (bf16) — no DRAM round trip.
- attn@v and the 1x1 proj are fused: M = w_proj @ blockdiag(attn) computed
  once in phase B; phase C is a single matmul over v + biased evac.
- One pairwise AllReduce of the [96, 4*96] gram block (includes sumsq).
"""

import sys

sys.path.insert(0, "/opt/trn_rl_repo")

import numpy as np

import concourse.bass as bass
import concourse.mybir as mybir
import concourse.tile as tile
from concourse import bacc
from concourse.bass_utils import run_bass_kernel_spmd

F32 = mybir.dt.float32
BF16 = mybir.dt.bfloat16
AF = mybir.ActivationFunctionType
OP = mybir.AluOpType
AX = mybir.AxisListType
NPBF16 = mybir.dt.np(BF16)
FP8 = mybir.dt.float8e4
NPFP8 = mybir.dt.np(FP8)
WQ_SCALE = 32.0
V_SCALE = 8.0
WM_SCALE = 128.0

DIM = 192
HEAD_DIM = 48
NH = 4
H = 256
W = 256
B = 4
N_CORES = 8
HALF = H // 2          # 128 output rows per shard
PR = HALF + 2          # 130 padded rows per shard
C3 = 3 * DIM           # 576
SCALE = HEAD_DIM ** -0.5
EPS = 1e-12

CT = [(0, 128), (128, 128), (256, 128), (384, 128), (512, 64)]
NITER = PR // 2        # 65 qkv row-pair iterations
NDW = NITER - 1        # 64 output row-pair iterations

# tap order used for the off-PE chains (kh, kw); kh=1 handled as half-rows
TAPS_FULL = [(0, 0), (0, 1), (0, 2), (2, 0), (2, 1), (2, 2)]
# bias_pack column indices
BQ, HBT, HBB = 0, 5, 10
BDW3, BDW4, BP0, BP1 = 15, 16, 17, 18
T0W, T0B = 19, 28      # 9 taps: TAPS_FULL + kh1 kw0..2
T1W, T1B = 29, 35      # 6 taps: TAPS_FULL
T2W, T2B = 36, 45      # 9 taps: TAPS_FULL + kh1 kw0..2
NBIAS = 46

_CACHED = {}


def _build_nc(repeat=1, no_cc=False, n_cores=N_CORES):
    nc = bacc.Bacc("TRN2", target_bir_lowering=False, debug=False,
                   enable_asserts=True, num_devices=n_cores)

    x_d = nc.dram_tensor("x_sh", [96, 2, PR, W], FP8, kind="ExternalInput").ap()
    wqkvT_d = nc.dram_tensor("wqkvT", [96, 2, C3], FP8, kind="ExternalInput").ap()
    wdw_d = nc.dram_tensor("wdw_diag", [128, 24 * 128], BF16,
                           kind="ExternalInput").ap()
    wprojT_d = nc.dram_tensor("wprojT", [DIM, DIM], BF16, kind="ExternalInput").ap()
    ident_d = nc.dram_tensor("ident", [128, 128], F32, kind="ExternalInput").ap()
    bias_d = nc.dram_tensor("bias_pack", [NBIAS, 128], F32, kind="ExternalInput").ap()
    mask_d = nc.dram_tensor("diag_mask", [96, 4 * 96], F32, kind="ExternalInput").ap()
    out_d = nc.dram_tensor("out_sh", [DIM, HALF, W], F32, kind="ExternalOutput").ap()

    with tile.TileContext(nc) as tc:
        for _rep in range(repeat):
            with (
                tc.tile_pool(name="const", bufs=1) as constp,
                tc.tile_pool(name="small", bufs=1) as smallp,
                tc.tile_pool(name="dram", bufs=1, space="DRAM") as dram,
            ):
                # ---- constants into SBUF ----
                wq8 = constp.tile([96, 2, C3], FP8)
                nc.sync.dma_start(wq8[:], wqkvT_d[:])
                wdw_sb = constp.tile([128, 24 * 128], BF16)
                nc.sync.dma_start(wdw_sb[:], wdw_d[:])
                wp_a = constp.tile([128, DIM], BF16)
                wp_b = constp.tile([64, DIM], BF16)
                nc.sync.dma_start(wp_a[:], wprojT_d[0:128, :])
                nc.sync.dma_start(wp_b[:], wprojT_d[128:192, :])
                ident = constp.tile([128, 128], F32)
                nc.sync.dma_start(ident[:], ident_d[:])
                ident_bf = constp.tile([128, 128], BF16)
                nc.scalar.copy(ident_bf[:], ident[:])
                bias_sb = constp.tile([128, NBIAS], F32)
                nc.sync.dma_start(bias_sb[:], bias_d.rearrange("r c -> c r"))
                mask_sb = constp.tile([96, 4 * 96], F32)
                nc.sync.dma_start(mask_sb[:], mask_d[:])

                def bc(col, n=128):
                    return bias_sb[0:n, col:col + 1]

                qkv_slots = [[constp.tile([128, 2, W + 2], BF16,
                                           name=f"qkvs{_rep}_{_s}_{_t}")
                              for _t in range(5)] for _s in range(3)]
                for _s in range(3):
                    for _t, (_c0, _nt) in enumerate(CT):
                        sbt = qkv_slots[_s][_t]
                        nc.gpsimd.memset(sbt[0:_nt, :, 0:1], 0.0)
                        nc.gpsimd.memset(sbt[0:_nt, :, W + 1:W + 2], 0.0)

                v8_slab = constp.tile([128, NDW, 2, 2, W], FP8)
                nc.gpsimd.memset(v8_slab[64:128, :, 1], 0.0)
                g_bin = dram.tile([96, 4 * 96], F32)
                g_bout = dram.tile([96, 4 * 96], F32)

                with (
                    tc.tile_pool(name="xin", bufs=2) as xpool,
                    tc.tile_pool(name="dwq", bufs=2) as dwqp,
                    tc.tile_pool(name="accs", bufs=2) as accp,
                    tc.tile_pool(name="sT", bufs=3) as sTpool,
                    tc.tile_pool(name="qkps", bufs=2, space="PSUM") as qkps,
                    tc.tile_pool(name="dwps", bufs=3, space="PSUM") as dwps,
                    tc.tile_pool(name="tps", bufs=2, space="PSUM") as tps,
                    tc.tile_pool(name="gram", bufs=1, space="PSUM") as gramp,
                ):
                    gram_ps = gramp.tile([96, 4 * 96], F32)
                    qkv_prev = None
                    first_gram = [True]

                    for j in range(NITER):
                        # ---- x rows 2j,2j+1 -> qkv conv ----
                        x8 = xpool.tile([96, 2, 2, W], FP8, tag="xa")
                        nc.sync.dma_start(x8[:], x_d[:, :, 2 * j:2 * j + 2, :])
                        qkv_cur = []
                        IS = 1.0 / WQ_SCALE
                        for t, (c0, nt) in enumerate(CT):
                            ps = qkps.tile([128, 2, W], F32, tag="qkps")
                            nc.tensor.matmul(
                                ps[0:nt], wq8[:, :, c0:c0 + nt], x8[:],
                                start=True, stop=True,
                                perf_mode=mybir.MatmulPerfMode.DoubleRow)
                            sb = qkv_slots[j % 3][t]
                            if j == 0:
                                nc.scalar.activation(sb[0:nt, 0, 1:W + 1],
                                                     ps[0:nt, 0], AF.Identity,
                                                     bias=bc(HBT + t, nt),
                                                     scale=IS)
                                nc.scalar.activation(sb[0:nt, 1, 1:W + 1],
                                                     ps[0:nt, 1], AF.Identity,
                                                     bias=bc(BQ + t, nt),
                                                     scale=IS)
                            elif j == NITER - 1:
                                nc.scalar.activation(sb[0:nt, 0, 1:W + 1],
                                                     ps[0:nt, 0], AF.Identity,
                                                     bias=bc(BQ + t, nt),
                                                     scale=IS)
                                nc.scalar.activation(sb[0:nt, 1, 1:W + 1],
                                                     ps[0:nt, 1], AF.Identity,
                                                     bias=bc(HBB + t, nt),
                                                     scale=IS)
                            else:
                                nc.scalar.activation(sb[0:nt, :, 1:W + 1],
                                                     ps[0:nt], AF.Identity,
                                                     bias=bc(BQ + t, nt),
                                                     scale=IS)
                            qkv_cur.append(sb)

                        if j >= 1:
                            i = j - 1
                            A, Bt = qkv_prev, qkv_cur

                            def dg(slot, nt=128):
                                return wdw_sb[0:nt, slot * 128:slot * 128 + nt]

                            def pe_taps(t, slotbase, dps, nt):
                                first = True
                                for kh, kw in TAPS_FULL:
                                    src = (A if kh == 0 else Bt)[t]
                                    nc.tensor.matmul(
                                        dps[0:nt, 0:2, :],
                                        dg(slotbase + kh * 3 + kw, nt),
                                        src[0:nt, 0:2, kw:kw + W],
                                        start=first, stop=False)
                                    first = False
                                for kw in range(3):
                                    nc.tensor.matmul(
                                        dps[0:nt, 0, :], dg(slotbase + 3 + kw, nt),
                                        A[t][0:nt, 1, kw:kw + W],
                                        start=False, stop=False)
                                    nc.tensor.matmul(
                                        dps[0:nt, 1, :], dg(slotbase + 3 + kw, nt),
                                        Bt[t][0:nt, 0, kw:kw + W],
                                        start=False, stop=(kw == 2))

                            # ---- t2: products on DVE (4x ts), adds on Pool ----
                            t2p = []
                            for idx, (kh, kw) in enumerate(TAPS_FULL):
                                src = (A if kh == 0 else Bt)[2]
                                tmp = accp.tile([128, 2, W], BF16,
                                                tag=f"t2p{idx}")
                                if idx == 0:
                                    nc.vector.tensor_scalar(
                                        tmp[:], src[:, 0:2, kw:kw + W],
                                        bc(T2W), bc(T2B),
                                        op0=OP.mult, op1=OP.add)
                                elif idx in (1, 2, 4):
                                    nc.gpsimd.tensor_scalar(
                                        tmp[:], src[:, 0:2, kw:kw + W],
                                        bc(T2W + idx), None, op0=OP.mult)
                                else:
                                    nc.vector.tensor_scalar(
                                        tmp[:], src[:, 0:2, kw:kw + W],
                                        bc(T2W + idx), None, op0=OP.mult)
                                t2p.append(tmp)
                            for kw in range(3):
                                tmp = accp.tile([128, 2, W], BF16,
                                                tag=f"t2h{kw}")
                                nc.vector.tensor_scalar(
                                    tmp[:, 0], A[2][:, 1, kw:kw + W],
                                    bc(T2W + 6 + kw), None, op0=OP.mult)
                                nc.vector.tensor_scalar(
                                    tmp[:, 1], Bt[2][:, 0, kw:kw + W],
                                    bc(T2W + 6 + kw), None, op0=OP.mult)
                                t2p.append(tmp)
                            # tree adds on Pool, in-place into product tiles
                            for a, b in ((0, 1), (2, 3), (4, 5), (6, 7)):
                                nc.gpsimd.tensor_tensor(
                                    t2p[a][:], t2p[a][:], t2p[b][:], op=OP.add)
                            nc.gpsimd.tensor_tensor(t2p[0][:], t2p[0][:],
                                                    t2p[2][:], op=OP.add)
                            nc.gpsimd.tensor_tensor(t2p[4][:], t2p[4][:],
                                                    t2p[6][:], op=OP.add)
                            nc.gpsimd.tensor_tensor(t2p[0][:], t2p[0][:],
                                                    t2p[4][:], op=OP.add)
                            dwq2 = dwqp.tile([128, 2, W], BF16, tag="dwq2")
                            nc.gpsimd.tensor_tensor(dwq2[:], t2p[0][:],
                                                    t2p[8][:], op=OP.add)

                            # ---- t3, t4 (v) fully on PE ----
                            dps3 = dwps.tile([128, 2, W], F32, tag="dwps")
                            pe_taps(3, 3, dps3, 128)  # slots 3..11
                            nc.scalar.activation(v8_slab[:, i, 0], dps3[:],
                                                 AF.Identity, bias=bc(BDW3),
                                                 scale=V_SCALE)
                            dps4 = dwps.tile([128, 2, W], F32, tag="dwps")
                            pe_taps(4, 12, dps4, 64)  # slots 12..20
                            nc.scalar.activation(v8_slab[0:64, i, 1], dps4[0:64],
                                                 AF.Identity, bias=bc(BDW4, 64),
                                                 scale=V_SCALE)

                            # ---- t1: kh=1 on PE, kh0/kh2 on ACT+DVE ----
                            dps1 = dwps.tile([128, 2, W], F32, tag="dwps")
                            for kw in range(3):
                                nc.tensor.matmul(dps1[:, 0, :], dg(kw),
                                                 A[1][:, 1, kw:kw + W],
                                                 start=(kw == 0), stop=(kw == 2))
                            for kw in range(3):
                                nc.tensor.matmul(dps1[:, 1, :], dg(kw),
                                                 Bt[1][:, 0, kw:kw + W],
                                                 start=(kw == 0), stop=(kw == 2))
                            acc1 = accp.tile([128, 2, W], BF16, tag="acc1")
                            nc.vector.tensor_scalar(
                                acc1[:], A[1][:, 0:2, 0:W], bc(T1W), bc(T1B),
                                op0=OP.mult, op1=OP.add)
                            t1p = []
                            for idx, (kh, kw) in enumerate(TAPS_FULL[1:], 1):
                                src = (A if kh == 0 else Bt)[1]
                                tmp = accp.tile([128, 2, W], BF16,
                                                tag=f"t1tmp{idx}")
                                if idx in (2, 4):  # two products on Pool
                                    nc.gpsimd.tensor_scalar(
                                        tmp[:], src[:, 0:2, kw:kw + W],
                                        bc(T1W + idx), None, op0=OP.mult)
                                else:
                                    nc.vector.tensor_scalar(
                                        tmp[:], src[:, 0:2, kw:kw + W],
                                        bc(T1W + idx), None, op0=OP.mult)
                                t1p.append(tmp)
                            # tree: 3 independent pair adds, then 2 levels
                            nc.vector.tensor_tensor(acc1[:], acc1[:], t1p[0][:],
                                                    op=OP.add)
                            nc.gpsimd.tensor_tensor(t1p[1][:], t1p[1][:],
                                                    t1p[2][:], op=OP.add)
                            nc.gpsimd.tensor_tensor(t1p[3][:], t1p[3][:],
                                                    t1p[4][:], op=OP.add)
                            nc.vector.tensor_tensor(acc1[:], acc1[:], t1p[1][:],
                                                    op=OP.add)
                            nc.vector.tensor_tensor(acc1[:], acc1[:], t1p[3][:],
                                                    op=OP.add)
                            dwq1 = dwqp.tile([128, 2, W], BF16, tag="dwq1")
                            nc.vector.scalar_tensor_tensor(
                                dwq1[:], dps1[:], 1.0, acc1[:],
                                op0=OP.mult, op1=OP.add)

                            # ---- t0: kh0 on PE, kh1/kh2 on DVE ----
                            dps0 = dwps.tile([128, 2, W], F32, tag="dwps")
                            for kw in range(3):
                                nc.tensor.matmul(dps0[:, 0:2, :], dg(21 + kw),
                                                 A[0][:, 0:2, kw:kw + W],
                                                 start=(kw == 0), stop=(kw == 2))
                            acc0 = accp.tile([128, 2, W], BF16, tag="acc0")
                            nc.vector.tensor_scalar(
                                acc0[:], Bt[0][:, 0:2, 0:W], bc(T0W + 3), bc(T0B),
                                op0=OP.mult, op1=OP.add)
                            t0p = []
                            for idx in (4, 5):  # kh2 kw1, kw2
                                tmp = accp.tile([128, 2, W], BF16,
                                                tag=f"t0tmp{idx}")
                                nc.vector.tensor_scalar(
                                    tmp[:], Bt[0][:, 0:2, idx - 3:idx - 3 + W],
                                    bc(T0W + idx), None, op0=OP.mult)
                                t0p.append(tmp)
                            for kw in range(3):
                                tmp = accp.tile([128, 2, W], BF16,
                                                tag=f"t0h{kw}")
                                nc.vector.tensor_scalar(
                                    tmp[:, 0], A[0][:, 1, kw:kw + W],
                                    bc(T0W + 6 + kw), None, op0=OP.mult)
                                nc.vector.tensor_scalar(
                                    tmp[:, 1], Bt[0][:, 0, kw:kw + W],
                                    bc(T0W + 6 + kw), None, op0=OP.mult)
                                t0p.append(tmp)
                            # tree adds
                            nc.vector.tensor_tensor(acc0[:], acc0[:], t0p[0][:],
                                                    op=OP.add)
                            nc.vector.tensor_tensor(t0p[1][:], t0p[1][:],
                                                    t0p[2][:], op=OP.add)
                            nc.vector.tensor_tensor(t0p[3][:], t0p[3][:],
                                                    t0p[4][:], op=OP.add)
                            nc.vector.tensor_tensor(acc0[:], acc0[:], t0p[1][:],
                                                    op=OP.add)
                            nc.vector.tensor_tensor(acc0[:], acc0[:], t0p[3][:],
                                                    op=OP.add)
                            dwq0 = dwqp.tile([128, 2, W], BF16, tag="dwq0")
                            nc.vector.scalar_tensor_tensor(
                                dwq0[:], dps0[:], 1.0, acc0[:],
                                op0=OP.mult, op1=OP.add)


                            # ---- transposes + gram (q/k tiles dwq0..2) ----
                            dwq = [dwq0, dwq1, dwq2]
                            for s in range(4):
                                row, hf = divmod(s, 2)
                                tp = tps.tile([128, 384], BF16, tag="tps")
                                for t in range(3):
                                    nc.tensor.transpose(
                                        tp[:, 128 * t:128 * t + 128],
                                        dwq[t][:, row, 128 * hf:128 * hf + 128],
                                        ident_bf[:])
                                sT = sTpool.tile([128, 384], BF16, tag="sT")
                                if s % 2 == 0:
                                    nc.vector.tensor_copy(sT[:], tp[:])
                                else:
                                    nc.scalar.activation(sT[:], tp[:], AF.Identity)
                                for h in range(NH):
                                    last = (i == NDW - 1 and s == 3 and h == NH - 1)
                                    nc.tensor.matmul(
                                        gram_ps[:, 96 * h:96 * h + 96],
                                        sT[:, 96 * h:96 * h + 96],
                                        sT[:, 96 * h:96 * h + 96],
                                        start=first_gram[0], stop=last,
                                        skip_group_check=True)
                                    first_gram[0] = False
                        qkv_prev = qkv_cur

                    # ---- phase B: collective, norms, softmax, fused M ----
                    gram_sb = smallp.tile([96, 4 * 96], F32)
                    nc.vector.tensor_copy(gram_sb[:], gram_ps[:])
                    nc.sync.dma_start(g_bin[:], gram_sb[:])
                    if no_cc:
                        nc.sync.dma_start(g_bout[:], g_bin[:])
                    else:
                        groups = [[2 * g, 2 * g + 1] for g in range(n_cores // 2)]
                        nc.gpsimd.collective_compute(
                            "AllReduce", OP.add, replica_groups=groups,
                            ins=[g_bin[:]], outs=[g_bout[:]])
                    g2 = smallp.tile([96, 4 * 96], F32)
                    nc.sync.dma_start(g2[:], g_bout[:])

                    gmask = smallp.tile([96, 4 * 96], F32)
                    nc.vector.tensor_tensor(gmask[:], g2[:], mask_sb[:],
                                            op=OP.mult)
                    ssq = smallp.tile([96, NH], F32)
                    nc.vector.tensor_reduce(
                        ssq[:], gmask[:].rearrange("p (b e) -> p b e", b=4),
                        AX.X, OP.add)
                    nrm = smallp.tile([96, NH], F32)
                    nc.scalar.sqrt(nrm[:], ssq[:])
                    nc.vector.tensor_scalar_max(nrm[:], nrm[:], EPS)
                    rn = smallp.tile([96, NH], F32)
                    nc.vector.reciprocal(rn[:], nrm[:])
                    rn8 = smallp.tile([48, 8], F32)  # cols: q h0..h3, k h0..h3
                    nc.sync.dma_start(rn8[:, 0:4], rn[0:48, :])
                    nc.sync.dma_start(rn8[:, 4:8], rn[48:96, :])

                    att = smallp.tile([48, 4 * 48], F32)
                    mxs = smallp.tile([48, NH], F32)
                    sm = smallp.tile([48, NH], F32)
                    rs = smallp.tile([48, NH], F32)
                    for h in range(NH):
                        sl = slice(48 * h, 48 * h + 48)
                        nc.vector.tensor_scalar(
                            att[:, sl], g2[0:48, 96 * h + 48:96 * h + 96],
                            rn8[:, h:h + 1], None, op0=OP.mult)
                        tp = tps.tile([128, 128], F32, tag="tps")
                        nc.tensor.transpose(tp[0:48, 0:48], att[:, sl],
                                            ident[0:48, 0:48])
                        gt = smallp.tile([48, 48], F32, tag="gt")
                        nc.scalar.activation(gt[:], tp[0:48, 0:48], AF.Identity,
                                             scale=rn8[:, 4 + h:5 + h])
                        tp2 = tps.tile([128, 128], F32, tag="tps")
                        nc.tensor.transpose(tp2[0:48, 0:48], gt[:],
                                            ident[0:48, 0:48])
                        nc.vector.tensor_copy(att[:, sl], tp2[0:48, 0:48])
                        nc.vector.tensor_reduce(mxs[:, h:h + 1], att[:, sl],
                                                AX.X, OP.max, negate=True)
                        nc.vector.tensor_scalar_mul(mxs[:, h:h + 1],
                                                    mxs[:, h:h + 1], SCALE)
                        nc.scalar.activation(att[:, sl], att[:, sl], AF.Exp,
                                             bias=mxs[:, h:h + 1], scale=SCALE)
                        nc.vector.tensor_reduce(sm[:, h:h + 1], att[:, sl],
                                                AX.X, OP.add)
                        nc.vector.reciprocal(rs[:, h:h + 1], sm[:, h:h + 1])
                        nc.vector.tensor_scalar_mul(att[:, sl], att[:, sl],
                                                    rs[:, h:h + 1])

                    att_bf = smallp.tile([48, 4 * 48], BF16)
                    nc.vector.tensor_copy(att_bf[:], att[:])
                    ablk_a = smallp.tile([128, DIM], BF16)
                    ablk_b = smallp.tile([64, DIM], BF16)
                    nc.gpsimd.memset(ablk_a[:], 0.0)
                    nc.gpsimd.memset(ablk_b[:], 0.0)
                    nc.vector.tensor_copy(ablk_a[0:48, 0:48], att_bf[:, 0:48])
                    nc.sync.dma_start(ablk_a[48:96, 48:96], att_bf[:, 48:96])
                    nc.sync.dma_start(ablk_a[96:128, 96:144], att_bf[0:32, 96:144])
                    nc.sync.dma_start(ablk_b[0:16, 96:144], att_bf[32:48, 96:144])
                    nc.sync.dma_start(ablk_b[16:64, 144:192], att_bf[:, 144:192])

                    mta_ps = tps.tile([128, DIM], F32, tag="tps")
                    nc.tensor.matmul(mta_ps[:], ablk_a[:, 0:128], wp_a[:],
                                     start=True, stop=False)
                    nc.tensor.matmul(mta_ps[:], ablk_b[:, 0:128], wp_b[:],
                                     start=False, stop=True)
                    mtb_ps = tps.tile([128, DIM], F32, tag="tps")
                    nc.tensor.matmul(mtb_ps[0:64], ablk_a[:, 128:192], wp_a[:],
                                     start=True, stop=False)
                    nc.tensor.matmul(mtb_ps[0:64], ablk_b[:, 128:192], wp_b[:],
                                     start=False, stop=True)
                    mt8 = smallp.tile([128, 2, DIM], FP8)
                    nc.gpsimd.memset(mt8[64:128, 1], 0.0)
                    nc.scalar.activation(mt8[:, 0], mta_ps[:], AF.Identity,
                                         scale=WM_SCALE)
                    nc.scalar.activation(mt8[0:64, 1], mtb_ps[0:64],
                                         AF.Identity, scale=WM_SCALE)

                # ---- phase C: out = M^T-weighted v + bias ----
                NB = 4  # row-pairs per out DMA batch
                with (
                    tc.tile_pool(name="outsb", bufs=2) as outsbp,
                    tc.tile_pool(name="prps", bufs=3, space="PSUM") as prps,
                ):
                    for ib in range(NDW // NB):
                        st0 = outsbp.tile([128, NB * 2, W], F32, tag="ob0")
                        st1 = outsbp.tile([64, NB * 2, W], F32, tag="ob1")
                        ISC = 1.0 / (V_SCALE * WM_SCALE)
                        for k in range(NB):
                            i = ib * NB + k
                            pp0 = prps.tile([128, 2, W], F32, tag="pr0")
                            nc.tensor.matmul(
                                pp0[:], mt8[:, :, 0:128], v8_slab[:, i],
                                start=True, stop=True,
                                perf_mode=mybir.MatmulPerfMode.DoubleRow)
                            pp1 = prps.tile([128, 2, W], F32, tag="pr1")
                            nc.tensor.matmul(
                                pp1[0:64], mt8[:, :, 128:192], v8_slab[:, i],
                                start=True, stop=True,
                                perf_mode=mybir.MatmulPerfMode.DoubleRow)
                            nc.scalar.activation(st0[:, 2 * k:2 * k + 2, :],
                                                 pp0[:], AF.Identity,
                                                 bias=bc(BP0), scale=ISC)
                            nc.scalar.activation(st1[:, 2 * k:2 * k + 2, :],
                                                 pp1[0:64], AF.Identity,
                                                 bias=bc(BP1, 64), scale=ISC)
                        r0 = ib * NB * 2
                        nc.sync.dma_start(
                            out_d[0:128, r0:r0 + NB * 2, :], st0[:])
                        nc.sync.dma_start(
                            out_d[128:192, r0:r0 + NB * 2, :], st1[:])

    nc.compile()
    return nc


def _get_nc(repeat=1, no_cc=False):
    key = (repeat, no_cc)
    if key not in _CACHED:
        _CACHED[key] = _build_nc(repeat, no_cc)
    return _CACHED[key]


def _perm():
    p = []
    for h in range(NH):
        p += list(range(48 * h, 48 * h + 48))
        p += list(range(192 + 48 * h, 192 + 48 * h + 48))
    p += list(range(384, 576))
    return np.array(p)


def _prep_inputs(x, w_qkv, b_qkv, w_dw, b_dw, w_proj, b_proj):
    x = np.asarray(x, np.float32)
    wq = np.asarray(w_qkv, np.float32)[:, :, 0, 0]        # [576, 192]
    bq = np.asarray(b_qkv, np.float32)
    wd = np.asarray(w_dw, np.float32)[:, 0]               # [576, 3, 3]
    bd = np.asarray(b_dw, np.float32)
    wp = np.asarray(w_proj, np.float32)[:, :, 0, 0]       # [192, 192]
    bp = np.asarray(b_proj, np.float32)

    perm = _perm()
    wq_p, bq_p, wd_p, bd_p = wq[perm], bq[perm], wd[perm], bd[perm]

    wqT = wq_p.T * WQ_SCALE                               # [192, 576]
    wqkvT8 = np.ascontiguousarray(
        np.stack([wqT[0:96], wqT[96:192]], axis=1)).astype(NPFP8)  # [96,2,576]
    wprojT = np.ascontiguousarray(wp.T).astype(NPBF16)    # [192, 192]
    ident = np.eye(128, dtype=np.float32)

    # diag blocks: slots 0-2 t1 kh1 kw0..2; 3-11 t3; 12-20 t4 (row-major kh,kw)
    wdw_diag = np.zeros((128, 24 * 128), np.float32)
    idx128 = np.arange(128)
    for kw in range(3):
        wdw_diag[idx128, kw * 128 + idx128] = wd_p[128:256, 1, kw]
        wdw_diag[idx128, (21 + kw) * 128 + idx128] = wd_p[0:128, 0, kw]
    for kh in range(3):
        for kw in range(3):
            s = 3 + kh * 3 + kw
            wdw_diag[idx128, s * 128 + idx128] = wd_p[384:512, kh, kw]
            s = 12 + kh * 3 + kw
            i64 = np.arange(64)
            wdw_diag[i64, s * 128 + i64] = wd_p[512:576, kh, kw]
    wdw_diag = wdw_diag.astype(NPBF16)

    def pack5(v):  # [576] -> [5, 128]
        o = np.zeros((5, 128), np.float32)
        for t, (c0, nt) in enumerate(CT):
            o[t, :nt] = v[c0:c0 + nt]
        return o

    taps9 = TAPS_FULL + [(1, 0), (1, 1), (1, 2)]
    bias_pack = np.zeros((NBIAS, 128), np.float32)
    bq5 = pack5(bq_p)
    bias_pack[BQ:BQ + 5] = bq5
    bias_pack[BDW3, :] = bd_p[384:512] * V_SCALE
    bias_pack[BDW4, :64] = bd_p[512:576] * V_SCALE
    bias_pack[BP0, :] = bp[0:128]
    bias_pack[BP1, :64] = bp[128:192]
    for idx, (kh, kw) in enumerate(taps9):
        bias_pack[T0W + idx, :] = wd_p[0:128, kh, kw]
        bias_pack[T2W + idx, :] = wd_p[256:384, kh, kw]
    for idx, (kh, kw) in enumerate(TAPS_FULL):
        bias_pack[T1W + idx, :] = wd_p[128:256, kh, kw]
    bias_pack[T0B, :] = bd_p[0:128]
    bias_pack[T1B, :] = bd_p[128:256]
    bias_pack[T2B, :] = bd_p[256:384]

    mask = np.zeros((96, 4 * 96), np.float32)
    for b in range(4):
        mask[np.arange(96), b * 96 + np.arange(96)] = 1.0

    xp = np.pad(x, ((0, 0), (0, 0), (1, 1), (0, 0)))      # [4, 192, 258, 256]
    in_maps = []
    for core in range(N_CORES):
        b, hf = divmod(core, 2)
        xsh = xp[b, :, hf * HALF:hf * HALF + PR, :]
        x_sh = np.ascontiguousarray(
            np.stack([xsh[0:96], xsh[96:192]], axis=1)).astype(NPFP8)
        bpk = bias_pack.copy()
        bpk[HBT:HBT + 5] = 0.0 if hf == 0 else bq5
        bpk[HBB:HBB + 5] = bq5 if hf == 0 else 0.0
        in_maps.append({
            "x_sh": x_sh, "wqkvT": wqkvT8, "wdw_diag": wdw_diag,
            "wprojT": wprojT, "ident": ident, "diag_mask": mask,
            "bias_pack": np.ascontiguousarray(bpk),
        })
    return in_maps


def kernel(x, w_qkv, b_qkv, w_dw, b_dw, w_proj, b_proj):
    nc = _get_nc()
    in_maps = _prep_inputs(x, w_qkv, b_qkv, w_dw, b_dw, w_proj, b_proj)
    res = run_bass_kernel_spmd(nc, in_maps, core_ids=list(range(N_CORES)))
    out = np.empty((B, DIM, H, W), np.float32)
    for core in range(N_CORES):
        b, hf = divmod(core, 2)
        out[b, :, hf * HALF:(hf + 1) * HALF, :] = res.results[core]["out_sh"]
    return out

